# revision 18
# baseline (speedup 1.0000x reference)
"""Trainium2 Bass kernel for nn_ConditionalSoftmax (sampled-softmax NLL loss).

Computes, for each batch row b:
    v_c   = vectors[cs[b]]                      # [D]
    h     = relu(v_c @ W1 + b1)                 # [H]
    logit = h @ W2 + b2                         # [V]
    nll_b = logsumexp(logit) - logit[v2s[ws[b]]]

Sharding: data-parallel over batch across 8 NeuronCores (1024 rows/core),
weights replicated.  Per core the work is a [1024,512]@[512,20480] fp8
DoubleRow matmul (~135us of PE time) plus exp+sum of all 21M logits.  No
single engine can absorb the exp (ACT runs 1 elem/cycle @1.2GHz = 180us
with per-instruction overheads), so every 2048-column PSUM pair is split
across THREE engine pipelines that each keep up with the PE's ~1.7us
refill:

 - psA (cols 0-1023, its own 2-bank PSUM tile): one Exp activation with
   accum_out, so the ScalarEngine computes exp AND its row-sum in a
   single instruction; the elementwise output goes to scratch nobody
   reads.  A separate PSUM tile per consumer matters: accum_out makes
   the dep-tracker treat the activation as a *writer* of its input
   tile, and any other engine reading the same tile serializes behind
   it (measured: the whole sweep collapsed to one consumer at a time).
 - psB (cols 1024-2047, its own 2-bank tile): a Schraudolph bit-trick
   exp on the VectorEngine.  One tensor_scalar computes
   i16 = round(x*(2^7*log2e/32) + (127-c)*2^7) straight out of PSUM;
   the bitpattern read as bf16 IS exp(x/32) to ~2% per element, with c
   chosen so the SUM is unbiased to ~1e-3.  Pairs p=0,1 convert
   directly into the per-t bf16 accumulator; even/odd pair groups
   (2,3)..(8,9) convert into the two halves of a shared [128,2048] i16
   buffer so ONE wide bf16 2x-mode add folds two pairs at once.  Half
   the adds run on the otherwise-idle GpSimd engine, keeping the DVE
   under the PE's refill time.

The vocab loop is OUTER (pair p, then batch tile t) so each W2 chunk is
needed ~14us after the previous one while DMA delivers one every
~1.4us: W2 streaming never stalls compute and there is no bulk-load
head.  Matmuls alternate kg per chunk (kg0 start / kg1 stop) so
LDWEIGHTS ping-pongs the two weight buffers; grouping all kg0 first
reloads the same buffer back-to-back and measurably drops the PE to
~259ns per 512-wide matmul (vs ~206 alternating).

The target logit is computed on the PE too: the host gathers the needed
W2 columns (same fp8 values the main matmul uses), one 128x128
DoubleRow matmul pair per batch tile, diagonal extracted by
identity-mask + reduce on the DVE.

Vocab is padded 20000->20480; every pad column contributes exactly
bf16(0x3F79)=0.97265625 through the trick path (its logits are exactly
0), so Ln(S - 480*0.97265625) via the activation bias corrects it for
free.  ~24 N=512 warmup matmuls keep the PE's HAM clock gate at 8/8
through the gather/phase-1 window (phase-1 otherwise runs at 1.2GHz);
a dummy Exp pulls the activation table load off the critical path.
"""
import numpy as np
import ml_dtypes

import concourse.bass as bass
import concourse.mybir as mybir
import concourse.tile as tile
from concourse import bacc, bass_utils
from concourse.bass import IndirectOffsetOnAxis, ts
from concourse.masks import make_identity

# Problem shapes (hardcoded per contest contract)
N_VOCAB = 50000
V = 20000
VPAD = 20480      # 40 chunks of 512
NCHUNK = 40
NPAIR = 10        # pairs of 4 chunks -> psA+psB [128,1024]x2 per batch tile
D = 300
DP = 384          # D padded to 3*128
NDC = 3           # contraction chunks for D
H = 512
NKG = 2           # DoubleRow contraction groups for H (256 each)
NHC = 4           # 128-row contraction chunks for H
B = 8192
NCORES = 8
BL = B // NCORES  # 1024 rows per core
NBT = BL // 128   # 8 batch tiles of 128 rows

W2_SCALE = 32.0   # host pre-scale of W2 into fp8 range; undone by Exp scale
TRICKW = 1024     # columns per pair handled by the DVE trick exp (psB)
LASTW = 544       # valid psB columns of the last pair (512 + 32; V exact)

# Schraudolph constants for bf16:  i16 = round(x*A + B), bitcast bf16.
# x is the PSUM logit (scaled by W2_SCALE); c centers the per-element
# ratio so E[approx/exp] = 1 over a uniform phase.
_C_SHIFT = 0.05629
A_TRICK = 128.0 * np.log2(np.e) / W2_SCALE          # 5.7707801...
B_TRICK = (127.0 - _C_SHIFT) * 128.0                # 16248.795
# every pad column (logit exactly 0) contributes bitcast(round(B_TRICK)):
PAD_VAL = float(
    np.array([int(np.round(B_TRICK))], dtype=np.int16).view(ml_dtypes.bfloat16)[0]
)
PAD_CORR = (VPAD - V) * PAD_VAL                     # 466.875

F32 = mybir.dt.float32
BF16 = mybir.dt.bfloat16
FP8 = mybir.dt.float8e4
I32 = mybir.dt.int32
I16 = mybir.dt.int16
AF = mybir.ActivationFunctionType
OP = mybir.AluOpType
DR = mybir.MatmulPerfMode.DoubleRow
AX = mybir.AxisListType.X

_BUILD_CACHE = {}


def _build(b2_nz: bool):
    key = (b2_nz,)
    if key in _BUILD_CACHE:
        return _BUILD_CACHE[key]

    nc = bacc.Bacc(
        "TRN2",
        target_bir_lowering=False,
        debug=False,
        num_devices=NCORES,
        num_swdge_queues=4,
    )

    # embedding rows pre-gathered AND pre-transposed on the host:
    # vctq[ki, c, b] = vectors[cs[b], c*128+ki] (bf16, zero-padded d)
    vctq = nc.dram_tensor("vctq", [128, NDC, BL], BF16, kind="ExternalInput").ap()
    w1 = nc.dram_tensor("w1", [DP, H], BF16, kind="ExternalInput").ap()
    b1c = nc.dram_tensor("b1c", [128, NHC], F32, kind="ExternalInput").ap()
    # W2 pre-scaled by W2_SCALE, fp8, padded to VPAD, laid out
    # w2q[c, ki, kg, ko, n] = W2s[kg*256 + ko*128 + ki, c*512 + n]
    w2q = nc.dram_tensor(
        "w2q", [NCHUNK, 128, NKG, 2, 512], FP8, kind="ExternalInput"
    ).ap()
    # host-gathered target columns of W2s (same fp8 values):
    # tgt8[ki, t, kg, ko, m] = W2s[kg*256+ko*128+ki, v2s[ws[t*128+m]]]
    tgt8 = nc.dram_tensor(
        "tgt8", [128, NBT, NKG, 2, 128], FP8, kind="ExternalInput"
    ).ap()
    if b2_nz:
        b2rep = nc.dram_tensor("b2rep", [128, VPAD], BF16, kind="ExternalInput").ap()
    out3 = nc.dram_tensor("out3", [128, 3, NBT], F32, kind="ExternalOutput").ap()

    with tile.TileContext(nc) as tc:
        with (
            tc.tile_pool(name="consts", bufs=1) as consts,
            tc.tile_pool(name="scr", bufs=2) as scrp,
            tc.tile_pool(name="i16s", bufs=10) as i16p,
            tc.tile_pool(name="ps", bufs=2, space="PSUM") as psm,
        ):
            # ---- constant / input loads: the pre-transposed embeddings
            # first (phase 1 waits on them), then the small weights, then
            # the W2 chunk stream.  All on the Sync sequencer. ----
            vcT = consts.tile([128, NDC, BL], BF16)    # v_c^T, d-major
            nc.sync.dma_start(vcT[:], vctq[:])
            w1sb = consts.tile([128, NDC, H], BF16)
            nc.sync.dma_start(w1sb[:], w1.rearrange("(c p) h -> p c h", p=128))
            b1sb = consts.tile([128, NHC], F32)
            nc.sync.dma_start(b1sb[:], b1c[:])
            tgtsb = consts.tile([128, NBT, NKG, 2, 128], FP8)
            nc.sync.dma_start(tgtsb[:], tgt8[:])
            w2all = consts.tile([128, NCHUNK, NKG, 2, 512], FP8)
            for p in range(NPAIR * 2):
                nc.sync.dma_start(
                    w2all[:, 2 * p : 2 * p + 2],
                    w2q[2 * p : 2 * p + 2].rearrange("c p kg ko n -> p c kg ko n"),
                )
            if b2_nz:
                b2rep_sb = consts.tile([128, VPAD], BF16)
                nc.sync.dma_start(b2rep_sb[:], b2rep[:])

            ident = consts.tile([128, 128], BF16)
            make_identity(nc, ident[:])

            # ---- PE warmup: identity matmuls spanning the gather window
            # so the HAM clock gate reaches (and keeps) 8/8 before phase-1.
            # Depends only on make_identity; the output is never read. ----
            wm = psm.tile([128, 1024], F32, tag="psA", name="warm")
            for i in range(80):
                nc.tensor.matmul(wm[:, 0:128], lhsT=ident[:], rhs=ident[:],
                                 start=True, stop=True)
            # dummy Exp: pull the activation-table load off the critical path
            dummy = consts.tile([128, 1], BF16)
            nc.scalar.activation(dummy[:], ident[:, 0:1], AF.Exp)

            # Long-lived activations
            hT8 = consts.tile([128, NKG, 2, BL], FP8)  # h^T fp8, DoubleRow layout
            finA = consts.tile([128, NBT, NPAIR + 1], F32)  # ACT accum partials
            accs = [
                consts.tile([128, 2 * TRICKW], BF16, name=f"acc{t}")
                for t in range(NBT)
            ]
            # SAB[:,0,:] = ACT partial total, [:,1,:] = trick total,
            # [:,2,:] = 32*target logit.  The Ln and subtractions happen
            # on the host (8192 values), saving the tail's activation
            # table swap and final vector ops.
            SAB = consts.tile([128, 3, NBT], F32)

            # ---- phase 1: W1 matmuls straight off the DMA-loaded vcT. ----
            def ph1_half(half):
                for gg in range(2):
                    pst = psm.tile([128, 1024], F32, tag="psB", name=f"pst{half}_{gg}")
                    for i in range(2):
                        hc = 2 * gg + i
                        for c in range(NDC):
                            nc.tensor.matmul(
                                pst[:, ts(i, 512)],
                                lhsT=w1sb[:, c, ts(hc, 128)],
                                rhs=vcT[:, c, ts(half, 512)],
                                start=(c == 0),
                                stop=(c == NDC - 1),
                            )
                    for i in range(2):
                        hc = 2 * gg + i
                        nc.vector.tensor_scalar(
                            out=hT8[:, hc // 2, hc % 2, ts(half, 512)],
                            in0=pst[:, ts(i, 512)],
                            scalar1=b1sb[:, hc : hc + 1],
                            scalar2=0.0,
                            op0=OP.add,
                            op1=OP.max,
                        )

            def tgt_diag(t):
                tg = psm.tile([128, 128], F32, tag="psB", name=f"tg{t}")
                for kg in range(NKG):
                    nc.tensor.matmul(
                        tg[:, 0:128],
                        lhsT=tgtsb[:, t, kg, :, :],
                        rhs=hT8[:, kg, :, ts(t, 128)],
                        start=(kg == 0),
                        stop=(kg == NKG - 1),
                        perf_mode=DR,
                    )
                prod = scrp.tile([128, 128], F32, tag="prod", name=f"prod{t}")
                nc.vector.tensor_mul(prod[:], tg[:, 0:128], ident[:])
                nc.vector.reduce_sum(out=SAB[:, 2, t : t + 1], in_=prod[:], axis=AX)

            # ---- phase 2: vocab-outer sweep.  Per (p,t): PE fills psA
            # (chunks 0,1) and psB (chunks 2,3); ACT exp+accums psA; DVE
            # trick-converts psB.  The last two sweeps are interleaved per
            # t so each batch tile's finals spread across the tail. ----
            tk2s = {}
            # deferred group-add schedule: group (2,3) folds at p=4,
            # (4,5) at p=6, (6,7) at p=8, (8,9) at p=9; engines alternate
            # by t parity (swapped per group) so neither the DVE nor the
            # ~4us-per-add gpsimd path falls behind the acc RMW chain.
            def fold_add(g, t):
                # gpsimd folds take ~4us: only where the acc chain has
                # slack -- mid-sweep alternating, and early block tiles.
                gps = (
                    (g == 3 and t % 2 == 0)
                    or (g == 5 and t % 2 == 1)
                    or (g == 7 and t < 6)
                    or (g == 9 and t < 4)
                )
                eng = nc.gpsimd if gps else nc.vector
                w = 2 * TRICKW if g < 8 else LASTW + TRICKW
                eng.tensor_add(
                    accs[t][:, :w], accs[t][:, :w],
                    tk2s[(g, t)].bitcast(BF16)[:, :w],
                )

            def pair(p, t):
                psA = psm.tile([128, 1024], F32, tag="psA", name=f"pa{t}_{p}")
                psB = psm.tile([128, 1024], F32, tag="psB", name=f"pb{t}_{p}")
                for c in range(4):
                    dst = psA if c < 2 else psB
                    n = 32 if 4 * p + c == NCHUNK - 1 else 512
                    for kg in range(NKG):
                        nc.tensor.matmul(
                            dst[:, (c % 2) * 512 : (c % 2) * 512 + n],
                            lhsT=hT8[:, kg, :, ts(t, 128)],
                            rhs=w2all[:, 4 * p + c, kg, :, 0:n],
                            start=(kg == 0),
                            stop=(kg == NKG - 1),
                            perf_mode=DR,
                        )
                if b2_nz:
                    nc.vector.tensor_add(
                        psA[:], psA[:], b2rep_sb[:, 2048 * p : 2048 * p + 1024]
                    )
                    nc.vector.tensor_add(
                        psB[:], psB[:],
                        b2rep_sb[:, 2048 * p + 1024 : 2048 * (p + 1)],
                    )
                es = scrp.tile([128, 1024], BF16, tag="escr", name=f"es{t}_{p}")
                nc.scalar.activation(
                    es[:],
                    psA[:],
                    AF.Exp,
                    scale=1.0 / W2_SCALE,
                    accum_out=finA[:, t, p : p + 1],
                )
                # trick-exp region: the valid width of psB (last pair holds
                # only 512+32 real columns -- V is covered exactly, no pad).
                bw = TRICKW if p < NPAIR - 1 else LASTW
                if p == NPAIR - 1 and t == NBT - 1:
                    # very last tile: psB goes through the ScalarEngine so
                    # the trailing trick chain is as short as possible.
                    es2 = scrp.tile([128, LASTW], BF16, tag="es2", name="es2")
                    nc.scalar.activation(
                        es2[:],
                        psB[:, 0:LASTW],
                        AF.Exp,
                        scale=1.0 / W2_SCALE,
                        accum_out=finA[:, t, NPAIR : NPAIR + 1],
                    )
                else:
                    if p < 2:
                        cdst = accs[t].bitcast(I16)[:, ts(p, TRICKW)]
                    elif p % 2 == 0:
                        tk2s[(p + 1, t)] = i16p.tile(
                            [128, 2 * TRICKW], I16, tag="i16s", name=f"tk{t}_{p}"
                        )
                        cdst = tk2s[(p + 1, t)][:, 0:TRICKW]
                    else:
                        cdst = tk2s[(p, t)][:, TRICKW : TRICKW + bw]
                    nc.vector.tensor_scalar(
                        out=cdst,
                        in0=psB[:, 0:bw],
                        scalar1=A_TRICK,
                        scalar2=B_TRICK,
                        op0=OP.mult,
                        op1=OP.add,
                    )
                # deferred folds
                if p in (4, 6, 8):
                    fold_add(p - 1, t)
                elif p == NPAIR - 1:
                    if t == NBT - 1:
                        w = TRICKW
                        nc.vector.tensor_add(
                            accs[t][:, :w], accs[t][:, :w],
                            tk2s[(9, t)].bitcast(BF16)[:, :w],
                        )
                    else:
                        fold_add(9, t)
                if p == NPAIR - 1:
                    # fold this tile's partial sums; split across ACT and
                    # DVE so the tail isn't one engine's serial chain.
                    if t % 2 == 1 or t == NBT - 1:
                        nc.vector.reduce_sum(
                            out=SAB[:, 1, t : t + 1], in_=accs[t][:], axis=AX
                        )
                    else:
                        fs = scrp.tile(
                            [128, 2 * TRICKW], BF16, tag="fscr", name=f"fs{t}"
                        )
                        nc.scalar.activation(
                            fs[:],
                            accs[t][:],
                            AF.Identity,
                            accum_out=SAB[:, 1, t : t + 1],
                        )
                    nc.vector.reduce_sum(
                        out=SAB[:, 0, t : t + 1], in_=finA[:, t, :], axis=AX
                    )

            nc.vector.memset(finA[:, :, NPAIR : NPAIR + 1], 0.0)
            ph1_half(0)
            for t in range(4):
                pair(0, t)
            ph1_half(1)
            for t in range(4, NBT):
                pair(0, t)
            for t in range(NBT):
                pair(1, t)
            for t in range(NBT):
                pair(2, t)
                tgt_diag(t)
            for p in range(3, 6):
                for t in range(NBT):
                    pair(p, t)
            for t in range(NBT):
                pair(6, t)
                pair(7, t)
                pair(8, t)
                pair(9, t)

            # ---- phase 3 (device side): just store the partials; the
            # host computes nll = log(Sa+Sb-corr) - tdiag/32 - b2[tgt]. ----
            nc.sync.dma_start(out3[:], SAB[:])

    nc.compile()
    _BUILD_CACHE[key] = nc
    return nc


def _prep_inputs(ws, cs, vectors, W1, b1, W2, b2, vector_to_support):
    ws = np.asarray(ws)
    cs = np.asarray(cs)
    vectors = np.asarray(vectors, dtype=np.float32)
    W1 = np.asarray(W1, dtype=np.float32)
    b1 = np.asarray(b1, dtype=np.float32)
    W2 = np.asarray(W2, dtype=np.float32)
    b2 = np.asarray(b2, dtype=np.float32)
    v2s = np.asarray(vector_to_support)

    b2_nz = bool(np.any(b2))

    w1p = np.zeros((DP, H), dtype=ml_dtypes.bfloat16)
    w1p[:D] = W1.astype(ml_dtypes.bfloat16)
    b1c = np.ascontiguousarray(b1.reshape(NHC, 128).T)

    # fp8 W2 (scaled), padded to VPAD:
    # w2q[c, ki, kg, ko, n] = W2s[kg*256 + ko*128 + ki, c*512 + n]
    w2s8 = (W2 * W2_SCALE).astype(ml_dtypes.float8_e4m3)
    w2pad = np.zeros((H, VPAD), dtype=ml_dtypes.float8_e4m3)
    w2pad[:, :V] = w2s8
    w2q = np.ascontiguousarray(
        w2pad.reshape(NKG, 2, 128, NCHUNK, 512).transpose(3, 2, 0, 1, 4)
    )

    shared = {
        "w1": w1p,
        "b1c": b1c,
        "w2q": w2q,
    }
    if b2_nz:
        b2pad = np.zeros((VPAD,), dtype=np.float32)
        b2pad[:V] = b2
        shared["b2rep"] = np.ascontiguousarray(
            np.broadcast_to(b2pad, (128, VPAD)).astype(ml_dtypes.bfloat16)
        )

    ws_sup = v2s[ws].astype(np.int64)

    in_maps = []
    b2ts = []
    for cidx in range(NCORES):
        sl = slice(cidx * BL, (cidx + 1) * BL)
        m = dict(shared)
        vcs = np.zeros((BL, DP), dtype=ml_dtypes.bfloat16)
        vcs[:, :D] = vectors[cs[sl]].astype(ml_dtypes.bfloat16)
        m["vctq"] = np.ascontiguousarray(vcs.reshape(BL, NDC, 128).transpose(2, 1, 0))
        wsup_c = ws_sup[sl]
        # tgt8[ki, t, kg, ko, m] = W2s[kg*256+ko*128+ki, wsup[t*128+m]]
        g = w2s8[:, wsup_c]  # [H, BL]
        m["tgt8"] = np.ascontiguousarray(
            g.reshape(NKG, 2, 128, NBT, 128).transpose(2, 3, 0, 1, 4)
        )
        b2ts.append(b2[wsup_c].astype(np.float64))
        in_maps.append(m)
    return in_maps, b2_nz, b2ts


def run(inputs: dict, trace: bool = False):
    """Run the SPMD kernel. Returns (output [B] fp32, BassKernelResults)."""
    in_maps, b2_nz, b2ts = _prep_inputs(**inputs)
    nc = _build(b2_nz)
    res = bass_utils.run_bass_kernel_spmd(
        nc, in_maps, core_ids=list(range(NCORES)), trace=trace
    )
    # out3 comes back [128, 3, NBT] with [p, ., t] = row t*128+p
    outs = []
    for c in range(NCORES):
        o = res.results[c]["out3"].astype(np.float64)
        S = o[:, 0, :] + o[:, 1, :]
        nllc = np.log(S) - o[:, 2, :] / W2_SCALE
        nllc = nllc.T.reshape(-1) - b2ts[c]
        outs.append(nllc)
    out = np.concatenate(outs).astype(np.float32)
    return out, res


def kernel(**inputs) -> np.ndarray:
    out, _ = run(inputs, trace=False)
    return out
